# revision 5
# baseline (speedup 1.0000x reference)
"""Trainium2 Bass kernel for masked BasicBlock (conv3x3+BN+ReLU, gated, x2, residual).

Data-parallel over batch: 8 images -> 8 NeuronCores. Per core:
  - NCHW image [64, 256, 256], processed in 8 row-strips of 32 output rows.
  - conv3x3 as 9 accumulated matmuls over C_in=64; taps K-packed to 128
    partitions via a column-shifted duplicate of the input (delta = +1 col),
    and chunk pairs (A|B = 4 consecutive rows) run concurrently on the two
    PE column groups via tile_position (0,0)/(0,64).
  - BN+ReLU on ScalarE from PSUM; gating masks broadcast to partitions via
    log2 doubling DMA chains from host-precomputed flat bf16 masks (upper
    mask half pre-shifted by 2 rows so one [128,512] DVE op covers a pair).
"""
import sys
import os

sys.path.insert(0, '/opt/trn_rl_repo')

import numpy as np
import ml_dtypes

BF16 = ml_dtypes.bfloat16

B, C, H, W = 8, 64, 256, 256
WP = W + 2           # padded row width
R = 32               # output rows per strip
NS = H // R          # strips
NP1 = (R + 4) // 4   # conv1 pairs per strip (h rows r0-1 .. r0+34)
NP2 = R // 4         # conv2 pairs per strip
XR = R + 6           # x rows per strip: [r0-2, r0+36)
HR = R + 4           # h rows per strip: [r0-1, r0+35)
PAD = 4              # zero rows padded above/below the flat masks

_CACHE = {}


def _build():
    import concourse.bacc as bacc_mod
    import concourse.tile as tile
    import concourse.mybir as mybir

    dt = mybir.dt
    nc = bacc_mod.Bacc()

    x_d = nc.dram_tensor("x", [C, H, W], dt.float32, kind="ExternalInput")
    gmx_d = nc.dram_tensor("gmx", [2, (H + 2 * PAD) * W], dt.bfloat16, kind="ExternalInput")
    gt_d = nc.dram_tensor("gt", [2, (H + 2 * PAD) * W], dt.bfloat16, kind="ExternalInput")
    wp1_d = nc.dram_tensor("wp1", [128, 3, 64], dt.bfloat16, kind="ExternalInput")
    ws1_d = nc.dram_tensor("ws1", [64, 3, 64], dt.bfloat16, kind="ExternalInput")
    wp2_d = nc.dram_tensor("wp2", [128, 3, 64], dt.bfloat16, kind="ExternalInput")
    ws2_d = nc.dram_tensor("ws2", [64, 3, 64], dt.bfloat16, kind="ExternalInput")
    sb1_d = nc.dram_tensor("sb1", [128, 2], dt.float32, kind="ExternalInput")  # scale1|bias1 dup'd
    sb2_d = nc.dram_tensor("sb2", [128, 2], dt.float32, kind="ExternalInput")
    o_d = nc.dram_tensor("o", [C, H, W], dt.float32, kind="ExternalOutput")

    with tile.TileContext(nc) as tc:
        with (
            tc.tile_pool(name="const", bufs=1) as cpool,
            tc.tile_pool(name="xs", bufs=2) as xpool,
            tc.tile_pool(name="hs", bufs=1) as hpool,
            tc.tile_pool(name="gm", bufs=2) as gmpool,
            tc.tile_pool(name="gt", bufs=2) as gtpool,
            tc.tile_pool(name="pair", bufs=4) as ppool,
            tc.tile_pool(name="ps1", bufs=2, space="PSUM") as ps1,
            tc.tile_pool(name="ps2", bufs=2, space="PSUM") as ps2,
        ):
            wp1 = cpool.tile([128, 3, 64], dt.bfloat16)
            ws1 = cpool.tile([64, 3, 64], dt.bfloat16)
            wp2 = cpool.tile([128, 3, 64], dt.bfloat16)
            ws2 = cpool.tile([64, 3, 64], dt.bfloat16)
            sb1 = cpool.tile([128, 2], dt.float32)
            sb2 = cpool.tile([128, 2], dt.float32)
            nc.sync.dma_start(wp1[:], wp1_d[:])
            nc.sync.dma_start(ws1[:], ws1_d[:])
            nc.sync.dma_start(wp2[:], wp2_d[:])
            nc.sync.dma_start(ws2[:], ws2_d[:])
            nc.sync.dma_start(sb1[:], sb1_d[:])
            nc.sync.dma_start(sb2[:], sb2_d[:])

            for s in range(NS):
                r0 = s * R
                # ---- x strip: T1 [128, XR, WP] bf16; lower=x padded, upper=x shifted +1 col
                T1 = xpool.tile([128, XR, WP], dt.bfloat16, tag="T1")
                lo_valid = max(0, 2 - r0 + r0)  # strip row idx of first valid image row
                # image rows covered: r0-2 .. r0+XR-3
                first = r0 - 2
                v0 = max(0, -first)            # leading invalid rows
                v1 = min(XR, H - first)        # end of valid rows
                nc.vector.memset(T1[0:64, :, 0:1], 0)
                nc.vector.memset(T1[0:64, :, 257:258], 0)
                if v0 > 0:
                    nc.vector.memset(T1[0:64, 0:v0, :], 0)
                if v1 < XR:
                    nc.vector.memset(T1[0:64, v1:XR, :], 0)
                nc.gpsimd.dma_start(T1[0:64, v0:v1, 1:257], x_d[:, first + v0:first + v1, :])
                # upper dup: cols 0..255 <- lower cols 1..256 shifted left by 1 => value at img col j+1?
                # upper[i, j] must equal lower[i, j+1]  (reads of upper use cols [0:256))
                nc.sync.dma_start(T1[64:128, :, 0:257], T1[0:64, :, 1:258])

                # ---- mask strips (host flat, upper pre-shifted by 2 rows)
                GM = gmpool.tile([128, HR, W], dt.bfloat16, tag="GM")
                GT = gtpool.tile([128, R, W], dt.bfloat16, tag="GT")
                gm_off = (r0 - 1 + PAD) * W
                gt_off = (r0 + PAD) * W
                nc.sync.dma_start(GM[0:1], gmx_d[0:1, gm_off:gm_off + HR * W].rearrange("p (r w) -> p r w", r=HR))
                nc.sync.dma_start(GM[64:65], gmx_d[1:2, gm_off:gm_off + HR * W].rearrange("p (r w) -> p r w", r=HR))
                nc.sync.dma_start(GT[0:1], gt_d[0:1, gt_off:gt_off + R * W].rearrange("p (r w) -> p r w", r=R))
                nc.sync.dma_start(GT[64:65], gt_d[1:2, gt_off:gt_off + R * W].rearrange("p (r w) -> p r w", r=R))
                p = 1
                while p < 64:
                    nc.sync.dma_start(GM[p:2 * p], GM[0:p])
                    nc.sync.dma_start(GM[64 + p:64 + 2 * p], GM[64:64 + p])
                    nc.sync.dma_start(GT[p:2 * p], GT[0:p])
                    nc.sync.dma_start(GT[64 + p:64 + 2 * p], GT[64:64 + p])
                    p *= 2

                # ---- h strip: H1 [128, HR, WP] bf16 (lower=h padded, upper=h shifted +1 col)
                H1 = hpool.tile([128, HR, WP], dt.bfloat16, tag="H1")
                nc.vector.memset(H1[0:64, :, 0:1], 0)
                nc.vector.memset(H1[0:64, :, 257:258], 0)

                # ---- conv1: 9 pairs of 4 h-rows
                for pq in range(NP1):
                    acc = ps1.tile([128, 512], dt.float32, tag="ps1")
                    lA = 4 * pq + 1  # T1 row idx of chunk-A first h row (h row y0 = r0-1+4p -> idx y0-(r0-2))
                    startA = True
                    startB = True
                    for dy in range(3):
                        ra = lA + dy - 1
                        nc.tensor.matmul(acc[0:64, :], wp1[:, dy, :], T1[:, ra:ra + 2, 0:256],
                                         start=startA, stop=False, tile_position=(0, 0), skip_group_check=True)
                        nc.tensor.matmul(acc[64:128, :], wp1[:, dy, :], T1[:, ra + 2:ra + 4, 0:256],
                                         start=startB, stop=False, tile_position=(0, 64), skip_group_check=True)
                        startA = startB = False
                    for dy in range(3):
                        ra = lA + dy - 1
                        nc.tensor.matmul(acc[0:64, :], ws1[:, dy, :], T1[0:64, ra:ra + 2, 2:258],
                                         start=False, stop=(dy == 2), tile_position=(0, 0), skip_group_check=True)
                        nc.tensor.matmul(acc[64:128, :], ws1[:, dy, :], T1[0:64, ra + 2:ra + 4, 2:258],
                                         start=False, stop=(dy == 2), tile_position=(0, 64), skip_group_check=True)
                    # BN + ReLU (ScalarE) -> bf16 staging
                    st = ppool.tile([128, 512], dt.bfloat16, tag="st")
                    nc.scalar.activation(st[:], acc[:], mybir.ActivationFunctionType.Relu,
                                         bias=sb1[:, 1:2], scale=sb1[:, 0:1])
                    # mask by dilated gate (DVE, bf16)
                    hp = ppool.tile([128, 512], dt.bfloat16, tag="hp")
                    nc.vector.tensor_tensor(hp[:].rearrange("p (r w) -> p r w", r=2),
                                            st[:].rearrange("p (r w) -> p r w", r=2),
                                            GM[:, 4 * pq:4 * pq + 2, :], mybir.AluOpType.mult)
                    # distribute into H1 (lower padded cols 1..256; upper = unshifted cols 0..255)
                    hr = 4 * pq
                    nc.sync.dma_start(H1[0:64, hr:hr + 2, 1:257], hp[0:64].rearrange("p (r w) -> p r w", r=2))
                    nc.sync.dma_start(H1[0:64, hr + 2:hr + 4, 1:257], hp[64:128].rearrange("p (r w) -> p r w", r=2))
                    nc.sync.dma_start(H1[64:128, hr:hr + 2, 0:256], hp[0:64].rearrange("p (r w) -> p r w", r=2))
                    nc.sync.dma_start(H1[64:128, hr + 2:hr + 4, 0:256], hp[64:128].rearrange("p (r w) -> p r w", r=2))

                # ---- conv2: 8 pairs of 4 output rows
                for q in range(NP2):
                    acc2 = ps2.tile([128, 512], dt.float32, tag="ps2")
                    mA = 4 * q + 1  # H1 row idx of chunk-A first out row (z0 = r0+4q -> idx z0-(r0-1))
                    startA = True
                    startB = True
                    for dy in range(3):
                        ra = mA + dy - 1
                        nc.tensor.matmul(acc2[0:64, :], wp2[:, dy, :], H1[:, ra:ra + 2, 0:256],
                                         start=startA, stop=False, tile_position=(0, 0), skip_group_check=True)
                        nc.tensor.matmul(acc2[64:128, :], wp2[:, dy, :], H1[:, ra + 2:ra + 4, 0:256],
                                         start=startB, stop=False, tile_position=(0, 64), skip_group_check=True)
                        startA = startB = False
                    for dy in range(3):
                        ra = mA + dy - 1
                        nc.tensor.matmul(acc2[0:64, :], ws2[:, dy, :], H1[0:64, ra:ra + 2, 2:258],
                                         start=False, stop=(dy == 2), tile_position=(0, 0), skip_group_check=True)
                        nc.tensor.matmul(acc2[64:128, :], ws2[:, dy, :], H1[0:64, ra + 2:ra + 4, 2:258],
                                         start=False, stop=(dy == 2), tile_position=(0, 64), skip_group_check=True)
                    # BN (Identity) -> f32
                    u2 = ppool.tile([128, 512], dt.float32, tag="u2")
                    nc.scalar.activation(u2[:], acc2[:], mybir.ActivationFunctionType.Identity,
                                         bias=sb2[:, 1:2], scale=sb2[:, 0:1])
                    # t = u2 * gate ; v = t + x ; out = relu(v)
                    t = ppool.tile([128, 512], dt.float32, tag="t")
                    nc.vector.tensor_tensor(t[:].rearrange("p (r w) -> p r w", r=2),
                                            u2[:].rearrange("p (r w) -> p r w", r=2),
                                            GT[:, 4 * q:4 * q + 2, :], mybir.AluOpType.mult)
                    lz = 4 * q + 2  # T1 row idx of out row z0
                    xr = ppool.tile([128, 2, 256], dt.bfloat16, tag="xr")
                    nc.sync.dma_start(xr[0:64], T1[0:64, lz:lz + 2, 1:257])
                    nc.sync.dma_start(xr[64:128], T1[0:64, lz + 2:lz + 4, 1:257])
                    v = ppool.tile([128, 512], dt.float32, tag="v")
                    nc.vector.tensor_tensor(v[:].rearrange("p (r w) -> p r w", r=2),
                                            t[:].rearrange("p (r w) -> p r w", r=2),
                                            xr[:], mybir.AluOpType.add)
                    ov = ppool.tile([128, 512], dt.float32, tag="ov")
                    nc.vector.tensor_scalar_max(ov[:], v[:], 0.0)
                    z0 = r0 + 4 * q
                    nc.sync.dma_start(o_d[:, z0:z0 + 2, :], ov[0:64].rearrange("p (r w) -> p r w", r=2))
                    nc.sync.dma_start(o_d[:, z0 + 2:z0 + 4, :], ov[64:128].rearrange("p (r w) -> p r w", r=2))
    nc.finalize()
    return nc


def _host_prep(gate, w1, scale1, bias1, w2, scale2, bias2):
    """Everything image-independent + per-image mask prep."""
    # weights: lhsT[ci, co] = w[co, ci, dy, dx]
    def pack(w):
        wt = np.transpose(w, (1, 0, 2, 3))  # [ci, co, 3, 3]
        wp = np.empty((128, 3, 64), np.float32)
        ws = np.empty((64, 3, 64), np.float32)
        for dy in range(3):
            wp[0:64, dy] = wt[:, :, dy, 0]     # dx=-1 (lower/K-low)
            wp[64:128, dy] = wt[:, :, dy, 1]   # dx=0  (upper/K-high, shifted dup)
            ws[:, dy] = wt[:, :, dy, 2]        # dx=+1 (single)
        return wp.astype(BF16), ws.astype(BF16)

    wp1, ws1 = pack(w1)
    wp2, ws2 = pack(w2)
    sb1 = np.stack([np.tile(scale1, 2), np.tile(bias1, 2)], axis=1).astype(np.float32)
    sb2 = np.stack([np.tile(scale2, 2), np.tile(bias2, 2)], axis=1).astype(np.float32)

    # masks: gmax = maxpool3x3(gate); flat padded [H+2*PAD, W]; row1 = shifted by 2 rows
    def flatten2(m):  # m [H, W] -> [2, (H+2*PAD)*W] bf16
        mp = np.zeros((H + 2 * PAD, W), np.float32)
        mp[PAD:PAD + H] = m
        m0 = mp.reshape(-1)
        m1 = np.zeros_like(m0)
        m1[:-2 * W] = m0[2 * W:]
        return np.stack([m0, m1]).astype(BF16)

    gmx_list, gt_list = [], []
    for b in range(B):
        g = gate[b, 0]
        gp = np.pad(g, 1)
        gm = np.zeros_like(g)
        for dy in range(3):
            for dx in range(3):
                np.maximum(gm, gp[dy:dy + H, dx:dx + W], out=gm)
        gmx_list.append(flatten2(gm))
        gt_list.append(flatten2(g))
    return wp1, ws1, wp2, ws2, sb1, sb2, gmx_list, gt_list


def kernel(x, gate, w1, scale1, bias1, w2, scale2, bias2):
    from concourse.bass_utils import run_bass_kernel_spmd

    x = np.asarray(x, np.float32)
    gate = np.asarray(gate, np.float32)
    wp1, ws1, wp2, ws2, sb1, sb2, gmx_list, gt_list = _host_prep(
        np.asarray(gate), np.asarray(w1, np.float32), np.asarray(scale1, np.float32),
        np.asarray(bias1, np.float32), np.asarray(w2, np.float32),
        np.asarray(scale2, np.float32), np.asarray(bias2, np.float32))

    if 'nc' not in _CACHE:
        _CACHE['nc'] = _build()
    nc = _CACHE['nc']

    in_maps = []
    for b in range(B):
        in_maps.append({
            "x": np.ascontiguousarray(x[b]),
            "gmx": gmx_list[b], "gt": gt_list[b],
            "wp1": wp1, "ws1": ws1, "wp2": wp2, "ws2": ws2,
            "sb1": sb1, "sb2": sb2,
        })
    res = run_bass_kernel_spmd(nc, in_maps, core_ids=list(range(B)))
    _CACHE['last_results'] = res
    out = np.stack([res.results[b]["o"] for b in range(B)], axis=0)
    return out


# revision 7
# speedup vs baseline: 1.0035x; 1.0035x over previous
"""Trainium2 Bass kernel for masked BasicBlock (conv3x3+BN+ReLU, gated, x2, residual).

Data-parallel over batch: 8 images -> 8 NeuronCores. Per core, NCHW [64,256,256]
in 8 row-strips of 32 output rows:
  - conv3x3 = 9 accumulated matmuls over C_in=64. Taps (dy=-1,dx)/(dy=+1,dx)
    are K-packed to 128 partitions via a 2-row-shifted duplicate of the input
    in partitions 64..127 (3 K=128 matmuls + 3 K=64 per chunk); chunk pairs
    (A|B = 4 consecutive rows) run concurrently on the two PE column groups
    via tile_position (0,0)/(0,64).
  - The 2-row shift also makes T1[0:128] directly usable as the residual pair.
  - Gating masks are broadcast to all partitions with K=1 ones-matmuls from a
    flat bf16 mask (PE->PSUM), not DMA chains.
  - BN(+ReLU) on ScalarE from PSUM; final relu on ScalarE; elementwise gating
    and residual on VectorE; strip-level staging tiles keep DMA count ~10/strip.
"""
import sys
import os

sys.path.insert(0, '/opt/trn_rl_repo')

import numpy as np
import ml_dtypes

BF16 = ml_dtypes.bfloat16

B, C, H, W = 8, 64, 256, 256
WP = W + 2           # padded row width
R = 32               # output rows per strip
NS = H // R          # strips
NP1 = (R + 4) // 4   # conv1 pairs per strip (h rows r0-1 .. r0+34)
NP2 = R // 4         # conv2 pairs per strip
XR = R + 6           # x rows per strip: [r0-2, r0+36)
HR = R + 4           # h rows per strip: [r0-1, r0+35)
PAD = 4              # zero rows padded above/below the flat masks
GMR = HR + 1         # gmax flat rows loaded per strip
GTR = R + 1          # gate flat rows loaded per strip

_CACHE = {}


def _build():
    import concourse.bacc as bacc_mod
    import concourse.tile as tile
    import concourse.mybir as mybir

    dt = mybir.dt
    nc = bacc_mod.Bacc()

    x_d = nc.dram_tensor("x", [C, H, W], dt.float32, kind="ExternalInput")
    gmx_d = nc.dram_tensor("gmx", [(H + 2 * PAD) * W], dt.bfloat16, kind="ExternalInput")
    gt_d = nc.dram_tensor("gt", [(H + 2 * PAD) * W], dt.bfloat16, kind="ExternalInput")
    wp1_d = nc.dram_tensor("wp1", [128, 3, 64], dt.bfloat16, kind="ExternalInput")
    ws1_d = nc.dram_tensor("ws1", [64, 3, 64], dt.bfloat16, kind="ExternalInput")
    wp2_d = nc.dram_tensor("wp2", [128, 3, 64], dt.bfloat16, kind="ExternalInput")
    ws2_d = nc.dram_tensor("ws2", [64, 3, 64], dt.bfloat16, kind="ExternalInput")
    sb1_d = nc.dram_tensor("sb1", [128, 2], dt.float32, kind="ExternalInput")
    sb2_d = nc.dram_tensor("sb2", [128, 2], dt.float32, kind="ExternalInput")
    o_d = nc.dram_tensor("o", [C, H, W], dt.float32, kind="ExternalOutput")

    with tile.TileContext(nc) as tc:
        with (
            tc.tile_pool(name="const", bufs=1) as cpool,
            tc.tile_pool(name="xs", bufs=2) as xpool,
            tc.tile_pool(name="hs", bufs=1) as hpool,
            tc.tile_pool(name="stage", bufs=2) as spool,
            tc.tile_pool(name="flat", bufs=2) as fpool,
            tc.tile_pool(name="pair", bufs=4) as ppool,
            tc.tile_pool(name="ps1", bufs=2, space="PSUM") as ps1,
            tc.tile_pool(name="ps2", bufs=2, space="PSUM") as ps2,
            tc.tile_pool(name="pmA", bufs=2, space="PSUM") as pmA,
            tc.tile_pool(name="pmB", bufs=2, space="PSUM") as pmB,
        ):
            wp1 = cpool.tile([128, 3, 64], dt.bfloat16)
            ws1 = cpool.tile([64, 3, 64], dt.bfloat16)
            wp2 = cpool.tile([128, 3, 64], dt.bfloat16)
            ws2 = cpool.tile([64, 3, 64], dt.bfloat16)
            sb1 = cpool.tile([128, 2], dt.float32)
            sb2 = cpool.tile([128, 2], dt.float32)
            ones = cpool.tile([1, 128], dt.bfloat16)
            nc.sync.dma_start(wp1[:], wp1_d[:])
            nc.sync.dma_start(ws1[:], ws1_d[:])
            nc.sync.dma_start(wp2[:], wp2_d[:])
            nc.sync.dma_start(ws2[:], ws2_d[:])
            nc.sync.dma_start(sb1[:], sb1_d[:])
            nc.sync.dma_start(sb2[:], sb2_d[:])
            nc.vector.memset(ones[:], 1.0)

            for s in range(NS):
                r0 = s * R
                # ---- x strip: T1 [128, XR, WP] bf16; lower=x padded; upper=x shifted +2 rows
                T1 = xpool.tile([128, XR, WP], dt.bfloat16, tag="T1")
                first = r0 - 2
                v0 = max(0, -first)
                v1 = min(XR, H - first)
                nc.vector.memset(T1[0:64, :, 0:1], 0)
                nc.vector.memset(T1[0:64, :, 257:258], 0)
                if v0 > 0:
                    nc.vector.memset(T1[0:64, 0:v0, :], 0)
                if v1 < XR:
                    nc.vector.memset(T1[0:64, v1:XR, :], 0)
                nc.gpsimd.dma_start(T1[0:64, v0:v1, 1:257], x_d[:, first + v0:first + v1, :])
                # upper[i] = lower[i+2]  (x shifted by +2 rows, same cols)
                nc.sync.dma_start(T1[64:128, 0:XR - 2, :], T1[0:64, 2:XR, :])

                # ---- flat mask strips (single partition, bf16)
                gmaxf = fpool.tile([1, GMR * W], dt.bfloat16, tag="gmaxf")
                gatef = fpool.tile([1, GTR * W], dt.bfloat16, tag="gatef")
                gm0 = (r0 + 3) * W
                gt0 = (r0 + PAD) * W
                nc.sync.dma_start(gmaxf[0:1, :], gmx_d[gm0:gm0 + GMR * W].unsqueeze(0))
                nc.sync.dma_start(gatef[0:1, :], gt_d[gt0:gt0 + GTR * W].unsqueeze(0))

                # ---- h strip + staging
                H1 = hpool.tile([128, HR, WP], dt.bfloat16, tag="H1")
                HP = spool.tile([128, NP1, 512], dt.bfloat16, tag="HP")
                OV = spool.tile([128, NP2, 512], dt.float32, tag="OV")
                nc.vector.memset(H1[:, :, 0:1], 0)
                nc.vector.memset(H1[:, :, 257:258], 0)

                # ---- conv1: 9 pairs of 4 h-rows
                for pq in range(NP1):
                    gm_ps = pmA.tile([128, 512], dt.float32, tag="pmA")
                    offA = 4 * pq * W
                    nc.tensor.matmul(gm_ps[0:64, :], ones[0:1, 0:64], gmaxf[0:1, offA:offA + 512],
                                     start=True, stop=True, tile_position=(0, 0), skip_group_check=True)
                    nc.tensor.matmul(gm_ps[64:128, :], ones[0:1, 64:128], gmaxf[0:1, offA + 512:offA + 1024],
                                     start=True, stop=True, tile_position=(0, 64), skip_group_check=True)

                    acc = ps1.tile([128, 512], dt.float32, tag="ps1")
                    i0 = 4 * pq + 1
                    for dx in range(3):
                        nc.tensor.matmul(acc[0:64, :], wp1[:, dx, :], T1[:, i0 - 1:i0 + 1, dx:dx + 256],
                                         start=(dx == 0), stop=False, tile_position=(0, 0), skip_group_check=True)
                        nc.tensor.matmul(acc[64:128, :], wp1[:, dx, :], T1[:, i0 + 1:i0 + 3, dx:dx + 256],
                                         start=(dx == 0), stop=False, tile_position=(0, 64), skip_group_check=True)
                    for dx in range(3):
                        nc.tensor.matmul(acc[0:64, :], ws1[:, dx, :], T1[0:64, i0:i0 + 2, dx:dx + 256],
                                         start=False, stop=(dx == 2), tile_position=(0, 0), skip_group_check=True)
                        nc.tensor.matmul(acc[64:128, :], ws1[:, dx, :], T1[0:64, i0 + 2:i0 + 4, dx:dx + 256],
                                         start=False, stop=(dx == 2), tile_position=(0, 64), skip_group_check=True)
                    st = ppool.tile([128, 512], dt.bfloat16, tag="st")
                    nc.scalar.activation(st[:], acc[:], mybir.ActivationFunctionType.Relu,
                                         bias=sb1[:, 1:2], scale=sb1[:, 0:1])
                    nc.vector.tensor_tensor(HP[:, pq, :], st[:], gm_ps[:], mybir.AluOpType.mult)

                # distribute HP -> H1 (lower = h, upper = h shifted +2 rows)
                # (DMA APs max 3 dims: one DMA per row-in-pair)
                h1v = H1[0:64, 0:4 * NP1, 1:257].rearrange("c (p rr) w -> c p rr w", p=NP1)
                h1u = H1[64:128, 2:2 + 4 * (NP1 - 1), 1:257].rearrange("c (p rr) w -> c p rr w", p=NP1 - 1)
                h1u2 = H1[64:128, 0:4 * NP1, 1:257].rearrange("c (p rr) w -> c p rr w", p=NP1)
                hpv = HP[:].rearrange("c p (rr w) -> c p rr w", rr=2)
                for rr in range(2):
                    nc.sync.dma_start(h1v[:, :, rr, :], hpv[0:64, :, rr, :])
                    nc.sync.dma_start(h1v[:, :, 2 + rr, :], hpv[64:128, :, rr, :])
                    nc.sync.dma_start(h1u[:, :, rr, :], hpv[0:64, 1:NP1, rr, :])
                    nc.sync.dma_start(h1u2[:, :, rr, :], hpv[64:128, :, rr, :])

                # ---- conv2: 8 pairs of 4 output rows
                for q in range(NP2):
                    gt_ps = pmB.tile([128, 512], dt.float32, tag="pmB")
                    offA = 4 * q * W
                    nc.tensor.matmul(gt_ps[0:64, :], ones[0:1, 0:64], gatef[0:1, offA:offA + 512],
                                     start=True, stop=True, tile_position=(0, 0), skip_group_check=True)
                    nc.tensor.matmul(gt_ps[64:128, :], ones[0:1, 64:128], gatef[0:1, offA + 512:offA + 1024],
                                     start=True, stop=True, tile_position=(0, 64), skip_group_check=True)

                    acc2 = ps2.tile([128, 512], dt.float32, tag="ps2")
                    m0 = 4 * q + 1
                    for dx in range(3):
                        nc.tensor.matmul(acc2[0:64, :], wp2[:, dx, :], H1[:, m0 - 1:m0 + 1, dx:dx + 256],
                                         start=(dx == 0), stop=False, tile_position=(0, 0), skip_group_check=True)
                        nc.tensor.matmul(acc2[64:128, :], wp2[:, dx, :], H1[:, m0 + 1:m0 + 3, dx:dx + 256],
                                         start=(dx == 0), stop=False, tile_position=(0, 64), skip_group_check=True)
                    for dx in range(3):
                        nc.tensor.matmul(acc2[0:64, :], ws2[:, dx, :], H1[0:64, m0:m0 + 2, dx:dx + 256],
                                         start=False, stop=(dx == 2), tile_position=(0, 0), skip_group_check=True)
                        nc.tensor.matmul(acc2[64:128, :], ws2[:, dx, :], H1[0:64, m0 + 2:m0 + 4, dx:dx + 256],
                                         start=False, stop=(dx == 2), tile_position=(0, 64), skip_group_check=True)
                    u2 = ppool.tile([128, 512], dt.float32, tag="u2")
                    nc.scalar.activation(u2[:], acc2[:], mybir.ActivationFunctionType.Identity,
                                         bias=sb2[:, 1:2], scale=sb2[:, 0:1])
                    t = ppool.tile([128, 512], dt.float32, tag="t")
                    nc.vector.tensor_tensor(t[:], u2[:], gt_ps[:], mybir.AluOpType.mult)
                    lz = 4 * q + 2
                    v = ppool.tile([128, 512], dt.float32, tag="v")
                    nc.vector.tensor_tensor(v[:].rearrange("p (r w) -> p r w", r=2),
                                            t[:].rearrange("p (r w) -> p r w", r=2),
                                            T1[:, lz:lz + 2, 1:257], mybir.AluOpType.add)
                    nc.scalar.activation(OV[:, q, :], v[:], mybir.ActivationFunctionType.Relu)

                ov = o_d[:, r0:r0 + R, :].rearrange("c (p rr) w -> c p rr w", p=NP2)
                ovv = OV[:].rearrange("c p (rr w) -> c p rr w", rr=2)
                for rr in range(2):
                    nc.sync.dma_start(ov[:, :, rr, :], ovv[0:64, :, rr, :])
                    nc.sync.dma_start(ov[:, :, 2 + rr, :], ovv[64:128, :, rr, :])
    nc.finalize()
    return nc


def _host_prep(gate, w1, scale1, bias1, w2, scale2, bias2):
    # weights: lhsT[ci, co] = w[co, ci, dy, dx]; K-pack dy=-1 (lower) with dy=+1 (upper)
    def pack(w):
        wt = np.transpose(w, (1, 0, 2, 3))  # [ci, co, dy, dx]
        wp = np.empty((128, 3, 64), np.float32)
        ws = np.empty((64, 3, 64), np.float32)
        for dx in range(3):
            wp[0:64, dx] = wt[:, :, 0, dx]
            wp[64:128, dx] = wt[:, :, 2, dx]
            ws[:, dx] = wt[:, :, 1, dx]
        return wp.astype(BF16), ws.astype(BF16)

    wp1, ws1 = pack(w1)
    wp2, ws2 = pack(w2)
    sb1 = np.stack([np.tile(scale1, 2), np.tile(bias1, 2)], axis=1).astype(np.float32)
    sb2 = np.stack([np.tile(scale2, 2), np.tile(bias2, 2)], axis=1).astype(np.float32)

    def flat_padded(m):
        mp = np.zeros((H + 2 * PAD, W), np.float32)
        mp[PAD:PAD + H] = m
        return mp.reshape(-1).astype(BF16)

    gmx_list, gt_list = [], []
    for b in range(B):
        g = gate[b, 0]
        gp = np.pad(g, 1)
        gm = np.zeros_like(g)
        for dy in range(3):
            for dx in range(3):
                np.maximum(gm, gp[dy:dy + H, dx:dx + W], out=gm)
        gmx_list.append(flat_padded(gm))
        gt_list.append(flat_padded(g))
    return wp1, ws1, wp2, ws2, sb1, sb2, gmx_list, gt_list


def kernel(x, gate, w1, scale1, bias1, w2, scale2, bias2):
    from concourse.bass_utils import run_bass_kernel_spmd

    x = np.asarray(x, np.float32)
    gate = np.asarray(gate, np.float32)
    wp1, ws1, wp2, ws2, sb1, sb2, gmx_list, gt_list = _host_prep(
        gate, np.asarray(w1, np.float32), np.asarray(scale1, np.float32),
        np.asarray(bias1, np.float32), np.asarray(w2, np.float32),
        np.asarray(scale2, np.float32), np.asarray(bias2, np.float32))

    if 'nc' not in _CACHE:
        _CACHE['nc'] = _build()
    nc = _CACHE['nc']

    in_maps = []
    for b in range(B):
        in_maps.append({
            "x": np.ascontiguousarray(x[b]),
            "gmx": gmx_list[b], "gt": gt_list[b],
            "wp1": wp1, "ws1": ws1, "wp2": wp2, "ws2": ws2,
            "sb1": sb1, "sb2": sb2,
        })
    res = run_bass_kernel_spmd(nc, in_maps, core_ids=list(range(B)))
    _CACHE['last_results'] = res
    out = np.stack([res.results[b]["o"] for b in range(B)], axis=0)
    return out


# revision 12
# speedup vs baseline: 5.9136x; 5.8929x over previous
"""Trainium2 Bass kernel for masked BasicBlock (conv3x3+BN+ReLU, gated, x2, residual).

Data-parallel over batch: 8 images -> 8 NeuronCores. Per core, NCHW [64,256,256]
in 8 row-strips of 32 output rows:
  - conv3x3 = 9 accumulated matmuls over C_in=64. Taps (dy=-1,dx)/(dy=+1,dx)
    are K-packed to 128 partitions via a 2-row-shifted duplicate of the input
    in partitions 64..127 (3 K=128 matmuls + 3 K=64 per chunk); chunk pairs
    (A|B = 4 consecutive rows) run concurrently on the two PE column groups
    via tile_position (0,0)/(0,64).
  - The 2-row shift also makes T1[0:128] directly usable as the residual pair.
  - Gating masks are broadcast to all partitions with K=1 ones-matmuls from a
    flat bf16 mask (PE->PSUM), not DMA chains.
  - BN(+ReLU) on ScalarE from PSUM; final relu on ScalarE; elementwise gating
    and residual on VectorE; strip-level staging tiles keep DMA count ~10/strip.
"""
import sys
import os

sys.path.insert(0, '/opt/trn_rl_repo')

import numpy as np
import ml_dtypes

BF16 = ml_dtypes.bfloat16

B, C, H, W = 8, 64, 256, 256
WP = W + 2           # padded row width
R = 32               # output rows per strip
NS = H // R          # strips
NP1 = (R + 4) // 4   # conv1 pairs per strip (h rows r0-1 .. r0+34)
NP2 = R // 4         # conv2 pairs per strip
XR = R + 6           # x rows per strip: [r0-2, r0+36)
HR = R + 4           # h rows per strip: [r0-1, r0+35)
PAD = 4              # zero rows padded above/below the flat masks
GMR = HR + 1         # gmax flat rows loaded per strip
GTR = R + 1          # gate flat rows loaded per strip

_CACHE = {}


def _build(iters=1):
    import concourse.bacc as bacc_mod
    import concourse.tile as tile
    import concourse.mybir as mybir

    dt = mybir.dt
    nc = bacc_mod.Bacc()

    x_d = nc.dram_tensor("x", [C, H, W], dt.float32, kind="ExternalInput")
    gmx_d = nc.dram_tensor("gmx", [(H + 2 * PAD) * W], dt.bfloat16, kind="ExternalInput")
    gt_d = nc.dram_tensor("gt", [(H + 2 * PAD) * W], dt.bfloat16, kind="ExternalInput")
    wp1_d = nc.dram_tensor("wp1", [128, 3, 64], dt.bfloat16, kind="ExternalInput")
    ws1_d = nc.dram_tensor("ws1", [64, 3, 64], dt.bfloat16, kind="ExternalInput")
    wp2_d = nc.dram_tensor("wp2", [128, 3, 64], dt.bfloat16, kind="ExternalInput")
    ws2_d = nc.dram_tensor("ws2", [64, 3, 64], dt.bfloat16, kind="ExternalInput")
    sb1_d = nc.dram_tensor("sb1", [128, 2], dt.float32, kind="ExternalInput")
    sb2_d = nc.dram_tensor("sb2", [128, 2], dt.float32, kind="ExternalInput")
    o_d = nc.dram_tensor("o", [C, H, W], dt.float32, kind="ExternalOutput")

    with tile.TileContext(nc) as tc:
        with (
            tc.tile_pool(name="const", bufs=1) as cpool,
            tc.tile_pool(name="xs", bufs=3) as xpool,
            tc.tile_pool(name="hs", bufs=1) as hpool,
            tc.tile_pool(name="stage", bufs=2) as spool,
            tc.tile_pool(name="ov", bufs=1) as ovpool,
            tc.tile_pool(name="flat", bufs=2) as fpool,
            tc.tile_pool(name="pair", bufs=3) as ppool,
            tc.tile_pool(name="ps1", bufs=2, space="PSUM") as ps1,
            tc.tile_pool(name="ps2", bufs=2, space="PSUM") as ps2,
            tc.tile_pool(name="pmA", bufs=2, space="PSUM") as pmA,
            tc.tile_pool(name="pmB", bufs=2, space="PSUM") as pmB,
        ):
            wp1 = cpool.tile([128, 3, 64], dt.bfloat16)
            ws1 = cpool.tile([64, 3, 64], dt.bfloat16)
            wp2 = cpool.tile([128, 3, 64], dt.bfloat16)
            ws2 = cpool.tile([64, 3, 64], dt.bfloat16)
            sb1 = cpool.tile([128, 2], dt.float32)
            sb2 = cpool.tile([128, 2], dt.float32)
            ones = cpool.tile([1, 128], dt.bfloat16)
            nc.sync.dma_start(wp1[:], wp1_d[:])
            nc.sync.dma_start(ws1[:], ws1_d[:])
            nc.sync.dma_start(wp2[:], wp2_d[:])
            nc.sync.dma_start(ws2[:], ws2_d[:])
            nc.sync.dma_start(sb1[:], sb1_d[:])
            nc.sync.dma_start(sb2[:], sb2_d[:])
            nc.vector.memset(ones[:], 1.0)

            for it_s in range(iters * NS):
                s = it_s % NS
                r0 = s * R
                # ---- x strip: T1 [128, XR, WP] bf16; lower=x padded; upper=x shifted +2 rows
                T1 = xpool.tile([128, XR, WP], dt.bfloat16, tag="T1")
                first = r0 - 2
                v0 = max(0, -first)
                v1 = min(XR, H - first)
                nc.vector.memset(T1[0:64, :, 0:1], 0)
                nc.vector.memset(T1[0:64, :, 257:258], 0)
                if v0 > 0:
                    nc.vector.memset(T1[0:64, 0:v0, :], 0)
                if v1 < XR:
                    nc.vector.memset(T1[0:64, v1:XR, :], 0)
                nc.gpsimd.dma_start(T1[0:64, v0:v1, 1:257], x_d[:, first + v0:first + v1, :])
                # upper[i] = lower[i+2]  (x shifted by +2 rows, same cols)
                nc.scalar.dma_start(T1[64:128, 0:XR - 2, :], T1[0:64, 2:XR, :])

                # ---- flat mask strips (single partition, bf16)
                gmaxf = fpool.tile([1, GMR * W], dt.bfloat16, tag="gmaxf")
                gatef = fpool.tile([1, GTR * W], dt.bfloat16, tag="gatef")
                gm0 = (r0 + 3) * W
                gt0 = (r0 + PAD) * W
                nc.sync.dma_start(gmaxf[0:1, :], gmx_d[gm0:gm0 + GMR * W].unsqueeze(0))
                nc.sync.dma_start(gatef[0:1, :], gt_d[gt0:gt0 + GTR * W].unsqueeze(0))

                # ---- h strip + staging
                H1 = hpool.tile([128, HR, WP], dt.bfloat16, tag="H1")
                HP = spool.tile([128, NP1, 512], dt.bfloat16, tag="HP")
                OV = ovpool.tile([128, NP2, 512], dt.float32, tag="OV")
                nc.vector.memset(H1[:, :, 0:1], 0)
                nc.vector.memset(H1[:, :, 257:258], 0)

                # ---- conv1: 9 pairs of 4 h-rows
                for pq in range(NP1):
                    gm_ps = pmA.tile([128, 512], dt.float32, tag="pmA")
                    offA = 4 * pq * W
                    nc.tensor.matmul(gm_ps[0:64, :], ones[0:1, 0:64], gmaxf[0:1, offA:offA + 512],
                                     start=True, stop=True, tile_position=(0, 0), skip_group_check=True)
                    nc.tensor.matmul(gm_ps[64:128, :], ones[0:1, 64:128], gmaxf[0:1, offA + 512:offA + 1024],
                                     start=True, stop=True, tile_position=(0, 64), skip_group_check=True)

                    acc = ps1.tile([128, 512], dt.float32, tag="ps1")
                    i0 = 4 * pq + 1
                    for dx in range(3):
                        nc.tensor.matmul(acc[0:64, :], wp1[:, dx, :], T1[:, i0 - 1:i0 + 1, dx:dx + 256],
                                         start=(dx == 0), stop=False, tile_position=(0, 0), skip_group_check=True)
                        nc.tensor.matmul(acc[64:128, :], wp1[:, dx, :], T1[:, i0 + 1:i0 + 3, dx:dx + 256],
                                         start=(dx == 0), stop=False, tile_position=(0, 64), skip_group_check=True)
                    for dx in range(3):
                        nc.tensor.matmul(acc[0:64, :], ws1[:, dx, :], T1[0:64, i0:i0 + 2, dx:dx + 256],
                                         start=False, stop=(dx == 2), tile_position=(0, 0), skip_group_check=True)
                        nc.tensor.matmul(acc[64:128, :], ws1[:, dx, :], T1[0:64, i0 + 2:i0 + 4, dx:dx + 256],
                                         start=False, stop=(dx == 2), tile_position=(0, 64), skip_group_check=True)
                    st = ppool.tile([128, 512], dt.bfloat16, tag="st")
                    nc.scalar.activation(st[:], acc[:], mybir.ActivationFunctionType.Relu,
                                         bias=sb1[:, 1:2], scale=sb1[:, 0:1])
                    nc.vector.tensor_tensor(HP[:, pq, :], st[:], gm_ps[:], mybir.AluOpType.mult)

                # distribute HP -> H1 (lower = h, upper = h shifted +2 rows)
                # (DMA APs max 3 dims: one DMA per row-in-pair)
                h1v = H1[0:64, 0:4 * NP1, 1:257].rearrange("c (p rr) w -> c p rr w", p=NP1)
                h1u = H1[64:128, 2:2 + 4 * (NP1 - 1), 1:257].rearrange("c (p rr) w -> c p rr w", p=NP1 - 1)
                h1u2 = H1[64:128, 0:4 * NP1, 1:257].rearrange("c (p rr) w -> c p rr w", p=NP1)
                hpv = HP[:].rearrange("c p (rr w) -> c p rr w", rr=2)
                for rr in range(2):
                    nc.sync.dma_start(h1v[:, :, rr, :], hpv[0:64, :, rr, :])
                    nc.sync.dma_start(h1v[:, :, 2 + rr, :], hpv[64:128, :, rr, :])
                    nc.sync.dma_start(h1u[:, :, rr, :], hpv[0:64, 1:NP1, rr, :])
                    nc.sync.dma_start(h1u2[:, :, rr, :], hpv[64:128, :, rr, :])

                # ---- conv2: 8 pairs of 4 output rows
                for q in range(NP2):
                    gt_ps = pmB.tile([128, 512], dt.float32, tag="pmB")
                    offA = 4 * q * W
                    nc.tensor.matmul(gt_ps[0:64, :], ones[0:1, 0:64], gatef[0:1, offA:offA + 512],
                                     start=True, stop=True, tile_position=(0, 0), skip_group_check=True)
                    nc.tensor.matmul(gt_ps[64:128, :], ones[0:1, 64:128], gatef[0:1, offA + 512:offA + 1024],
                                     start=True, stop=True, tile_position=(0, 64), skip_group_check=True)

                    acc2 = ps2.tile([128, 512], dt.float32, tag="ps2")
                    m0 = 4 * q + 1
                    for dx in range(3):
                        nc.tensor.matmul(acc2[0:64, :], wp2[:, dx, :], H1[:, m0 - 1:m0 + 1, dx:dx + 256],
                                         start=(dx == 0), stop=False, tile_position=(0, 0), skip_group_check=True)
                        nc.tensor.matmul(acc2[64:128, :], wp2[:, dx, :], H1[:, m0 + 1:m0 + 3, dx:dx + 256],
                                         start=(dx == 0), stop=False, tile_position=(0, 64), skip_group_check=True)
                    for dx in range(3):
                        nc.tensor.matmul(acc2[0:64, :], ws2[:, dx, :], H1[0:64, m0:m0 + 2, dx:dx + 256],
                                         start=False, stop=(dx == 2), tile_position=(0, 0), skip_group_check=True)
                        nc.tensor.matmul(acc2[64:128, :], ws2[:, dx, :], H1[0:64, m0 + 2:m0 + 4, dx:dx + 256],
                                         start=False, stop=(dx == 2), tile_position=(0, 64), skip_group_check=True)
                    u2 = ppool.tile([128, 512], dt.float32, tag="u2")
                    nc.scalar.activation(u2[:], acc2[:], mybir.ActivationFunctionType.Identity,
                                         bias=sb2[:, 1:2], scale=sb2[:, 0:1])
                    t = ppool.tile([128, 512], dt.float32, tag="t")
                    nc.vector.tensor_tensor(t[:], u2[:], gt_ps[:], mybir.AluOpType.mult)
                    lz = 4 * q + 2
                    v = ppool.tile([128, 512], dt.float32, tag="v")
                    nc.vector.tensor_tensor(v[:].rearrange("p (r w) -> p r w", r=2),
                                            t[:].rearrange("p (r w) -> p r w", r=2),
                                            T1[:, lz:lz + 2, 1:257], mybir.AluOpType.add)
                    nc.scalar.activation(OV[:, q, :], v[:], mybir.ActivationFunctionType.Relu)

                ov = o_d[:, r0:r0 + R, :].rearrange("c (p rr) w -> c p rr w", p=NP2)
                ovv = OV[:].rearrange("c p (rr w) -> c p rr w", rr=2)
                for rr in range(2):
                    nc.scalar.dma_start(ov[:, :, rr, :], ovv[0:64, :, rr, :])
                    nc.scalar.dma_start(ov[:, :, 2 + rr, :], ovv[64:128, :, rr, :])
    nc.finalize()
    return nc


def _host_prep(gate, w1, scale1, bias1, w2, scale2, bias2):
    # weights: lhsT[ci, co] = w[co, ci, dy, dx]; K-pack dy=-1 (lower) with dy=+1 (upper)
    def pack(w):
        wt = np.transpose(w, (1, 0, 2, 3))  # [ci, co, dy, dx]
        wp = np.empty((128, 3, 64), np.float32)
        ws = np.empty((64, 3, 64), np.float32)
        for dx in range(3):
            wp[0:64, dx] = wt[:, :, 0, dx]
            wp[64:128, dx] = wt[:, :, 2, dx]
            ws[:, dx] = wt[:, :, 1, dx]
        return wp.astype(BF16), ws.astype(BF16)

    wp1, ws1 = pack(w1)
    wp2, ws2 = pack(w2)
    sb1 = np.stack([np.tile(scale1, 2), np.tile(bias1, 2)], axis=1).astype(np.float32)
    sb2 = np.stack([np.tile(scale2, 2), np.tile(bias2, 2)], axis=1).astype(np.float32)

    def flat_padded(m):
        mp = np.zeros((H + 2 * PAD, W), np.float32)
        mp[PAD:PAD + H] = m
        return mp.reshape(-1).astype(BF16)

    gmx_list, gt_list = [], []
    for b in range(B):
        g = gate[b, 0]
        gp = np.pad(g, 1)
        gm = np.zeros_like(g)
        for dy in range(3):
            for dx in range(3):
                np.maximum(gm, gp[dy:dy + H, dx:dx + W], out=gm)
        gmx_list.append(flat_padded(gm))
        gt_list.append(flat_padded(g))
    return wp1, ws1, wp2, ws2, sb1, sb2, gmx_list, gt_list


def kernel(x, gate, w1, scale1, bias1, w2, scale2, bias2):
    from concourse.bass_utils import run_bass_kernel_spmd

    x = np.asarray(x, np.float32)
    gate = np.asarray(gate, np.float32)
    wp1, ws1, wp2, ws2, sb1, sb2, gmx_list, gt_list = _host_prep(
        gate, np.asarray(w1, np.float32), np.asarray(scale1, np.float32),
        np.asarray(bias1, np.float32), np.asarray(w2, np.float32),
        np.asarray(scale2, np.float32), np.asarray(bias2, np.float32))

    if 'nc' not in _CACHE:
        _CACHE['nc'] = _build()
    nc = _CACHE['nc']

    in_maps = []
    for b in range(B):
        in_maps.append({
            "x": np.ascontiguousarray(x[b]),
            "gmx": gmx_list[b], "gt": gt_list[b],
            "wp1": wp1, "ws1": ws1, "wp2": wp2, "ws2": ws2,
            "sb1": sb1, "sb2": sb2,
        })
    res = run_bass_kernel_spmd(nc, in_maps, core_ids=list(range(B)))
    _CACHE['last_results'] = res
    out = np.stack([res.results[b]["o"] for b in range(B)], axis=0)
    return out
